# revision 2
# baseline (speedup 1.0000x reference)
"""DirichletLoss kernel for 8 trn2 NeuronCores.

Math: per graph b, per channel d:
    de[d] = f_d^T L f_d  with f = row-normalized h.
A scalar equals its transpose, so f_d^T L f_d == f_d^T L^T f_d. We exploit
this to compute Mf = L^T @ f with L in its NATURAL row-major layout as the
matmul stationary operand (lhsT[K=i, M=j] = L[i, j]) and f (natural layout)
as the moving operand. No transpose of the big L matrices anywhere.

Sharding: graph b -> core b. Each core handles laplacian_s[b] and
laplacian_t[b] (16 MiB each), streaming them through the PE while
accumulating Mf in PSUM, then a multiply-reduce produces a [128, 2]
partial per core. Host finishes the (tiny) masked mean.
"""

import numpy as np

import concourse.bacc as bacc
import concourse.bass as bass
import concourse.mybir as mybir
import concourse.tile as tile
from concourse.bass_utils import run_bass_kernel_spmd

B = 8
N = 2048
D = 64
P = 128
NCHUNK = N // P  # 16
F32 = mybir.dt.float32
BF16 = mybir.dt.bfloat16

# --- tunables -------------------------------------------------------------
SLAB_ROWS = 256          # rows of L per DMA (must be multiple of 128)
SLAB_BUFS = 3            # slab pool double/triple buffering
# --------------------------------------------------------------------------


def _emit_body(nc, tc, pools, aps, variant="full"):
    fpool, slabpool, psumpool, smallpool, outpool = pools
    Ls, hs, Lt, ht, out = aps
    n_blk = SLAB_ROWS // P

    out_sb = outpool.tile([P, 2], F32, tag="out_sb")

    for side, (L_ap, h_ap) in enumerate(((Ls, hs), (Lt, ht))):
        # ---- load h, packed: f_raw[p, k*64+d] = h[k*128+p, d] ----
        f_raw = fpool.tile([P, NCHUNK * D], F32, tag="f_raw")
        nc.sync.dma_start(
            out=f_raw[:], in_=h_ap.rearrange("(k p) d -> p k d", p=P)
        )

        # ---- row L2 norms per (p, k) segment ----
        sq = smallpool.tile([P, NCHUNK * D], F32, tag="sq")
        nc.scalar.square(sq[:], f_raw[:])
        ss = smallpool.tile([P, NCHUNK], F32, tag="ss")
        nc.vector.reduce_sum(
            out=ss[:],
            in_=sq[:].rearrange("p (k d) -> p k d", d=D),
            axis=mybir.AxisListType.X,
        )
        nrm = smallpool.tile([P, NCHUNK], F32, tag="nrm")
        nc.scalar.sqrt(nrm[:], ss[:])
        nc.vector.tensor_scalar_max(nrm[:], nrm[:], 1e-12)
        inv = smallpool.tile([P, NCHUNK], F32, tag="inv")
        nc.vector.reciprocal(inv[:], nrm[:])

        # ---- f = h / max(||h||, eps), per chunk ----
        f_all = fpool.tile([P, NCHUNK * D], F32, tag="f_all")
        for k in range(NCHUNK):
            nc.scalar.mul(
                f_all[:, k * D : (k + 1) * D],
                f_raw[:, k * D : (k + 1) * D],
                inv[:, k : k + 1],
            )

        # ---- Mf = L^T @ f, accumulated over row-slabs ----
        mm_dt = BF16 if variant == "bf16" else F32
        f_mm = f_all
        if variant == "bf16":
            f_mm = fpool.tile([P, NCHUNK * D], BF16, tag="f_bf16")
            nc.vector.tensor_copy(f_mm[:], f_all[:])

        psum = psumpool.tile([P, NCHUNK * D], F32, tag="psum")
        pe_slab = None
        if variant == "pe":
            pe_slab = slabpool.tile([P, n_blk * N], F32, tag="slab")
            nc.sync.dma_start(
                out=pe_slab[:],
                in_=L_ap[0:SLAB_ROWS, :].rearrange("(n p) c -> p n c", p=P),
            )
        for g in range(N // SLAB_ROWS):
            if variant == "pe":
                slab = pe_slab
            else:
                slab = slabpool.tile([P, n_blk * N], mm_dt, tag="slab")
                src = L_ap[g * SLAB_ROWS : (g + 1) * SLAB_ROWS, :].rearrange(
                    "(n p) c -> p n c", p=P
                )
                if variant == "bf16":
                    # dtype cast during DMA is SWDGE-only
                    nc.gpsimd.dma_start(out=slab[:], in_=src)
                else:
                    nc.sync.dma_start(out=slab[:], in_=src)
            if variant == "dma":
                continue
            for n in range(n_blk):
                k = g * n_blk + n  # global contraction chunk
                for j in range(NCHUNK):
                    nc.tensor.matmul(
                        psum[:, j * D : (j + 1) * D],
                        slab[:, n * N + j * P : n * N + (j + 1) * P],
                        f_mm[:, k * D : (k + 1) * D],
                        # PSUM "pending zero" works at bank (2 KiB)
                        # granularity: start only on the first matmul
                        # touching each bank (j=0 and j=8 at k=0);
                        # later first-writes to other j-slices of the
                        # bank overwrite-where-unwritten automatically.
                        start=(k == 0 and j % 8 == 0),
                        stop=(k == NCHUNK - 1 and j % 8 == 7),
                    )
        if variant == "dma":
            # give psum a defined value so the epilogue is valid
            nc.vector.memset(psum[:], 0.0)

        # ---- r[p] = sum_{k,d} f * Mf ----
        # (tensor_tensor_reduce hard-crashes this HW/ucode build;
        # use separate multiply + reduce instead)
        tmp = smallpool.tile([P, NCHUNK * D], F32, tag="ttr_tmp")
        nc.vector.tensor_tensor(
            out=tmp[:], in0=psum[:], in1=f_all[:], op=mybir.AluOpType.mult
        )
        nc.vector.reduce_sum(
            out=out_sb[:, side : side + 1],
            in_=tmp[:],
            axis=mybir.AxisListType.X,
        )

    nc.sync.dma_start(out=out[:], in_=out_sb[:])


def build_program(reps=1, variant="full"):
    nc = bacc.Bacc(trn_type="TRN2")

    Ls = nc.declare_dram_parameter("Ls", [N, N], F32, isOutput=False)
    hs = nc.declare_dram_parameter("hs", [N, D], F32, isOutput=False)
    Lt = nc.declare_dram_parameter("Lt", [N, N], F32, isOutput=False)
    ht = nc.declare_dram_parameter("ht", [N, D], F32, isOutput=False)
    out = nc.declare_dram_parameter("out", [P, 2], F32, isOutput=True)
    aps = (Ls, hs, Lt, ht, out)

    with tile.TileContext(nc) as tc:
        with (
            tc.tile_pool(name="fpool", bufs=2) as fpool,
            tc.tile_pool(name="slab", bufs=SLAB_BUFS) as slabpool,
            tc.tile_pool(name="psum", bufs=2, space="PSUM") as psumpool,
            tc.tile_pool(name="small", bufs=2) as smallpool,
            tc.tile_pool(name="outp", bufs=2) as outpool,
        ):
            pools = (fpool, slabpool, psumpool, smallpool, outpool)
            if reps == 1:
                _emit_body(nc, tc, pools, aps, variant)
            else:
                with tc.For_i(0, reps, 1):
                    _emit_body(nc, tc, pools, aps, variant)

    nc.compile()
    return nc


_CACHED_NC = None


def _get_nc():
    global _CACHED_NC
    if _CACHED_NC is None:
        _CACHED_NC = build_program()
    return _CACHED_NC


def _shard_inputs(inputs):
    lap_s = np.ascontiguousarray(np.asarray(inputs["laplacian_s"], dtype=np.float32))
    lap_t = np.ascontiguousarray(np.asarray(inputs["laplacian_t"], dtype=np.float32))
    h_s = np.ascontiguousarray(np.asarray(inputs["h_s"], dtype=np.float32))
    h_t = np.ascontiguousarray(np.asarray(inputs["h_t"], dtype=np.float32))
    return [
        {
            "Ls": lap_s[b * N : (b + 1) * N],
            "hs": h_s[b * N : (b + 1) * N],
            "Lt": lap_t[b * N : (b + 1) * N],
            "ht": h_t[b * N : (b + 1) * N],
        }
        for b in range(B)
    ]


def _finish(core_outs, inputs):
    has_s = np.asarray(inputs["has_laplacian_s"]).astype(bool)
    has_t = np.asarray(inputs["has_laplacian_t"]).astype(bool)
    d_s = np.empty(B, dtype=np.float64)
    d_t = np.empty(B, dtype=np.float64)
    for b in range(B):
        o = np.asarray(core_outs[b], dtype=np.float64)
        d_s[b] = o[:, 0].sum() / D
        d_t[b] = o[:, 1].sum() / D
    per_graph = 0.5 * (d_s + d_t)
    valid = np.logical_and(has_s, has_t)
    count = valid.sum()
    total = per_graph[valid].sum()
    value = total / max(count, 1.0) if count > 0 else 0.0
    return np.array(value, dtype=np.float32)


def _run(inputs, trace=False, tmpdir=None):
    nc = _get_nc()
    in_maps = _shard_inputs(inputs)
    res = run_bass_kernel_spmd(nc, in_maps, list(range(B)), trace=trace, tmpdir=tmpdir)
    out = _finish([res.results[b]["out"] for b in range(B)], inputs)
    return out, res


def kernel(**inputs):
    out, _ = _run(inputs, trace=False)
    return out



# revision 5
# speedup vs baseline: 1.9688x; 1.9688x over previous
"""DirichletLoss kernel for 8 trn2 NeuronCores.

Math: per graph b, per channel d:
    de[d] = f_d^T L f_d  with f = row-normalized h.

Layout strategy (v2): make L the MOVING matmul operand so each matmul
streams N=512 columns (vs 64 in v1), and use float32r (single-pass fp32,
1 cycle/row at N>=256) instead of 4-pass fp32. The stationary operand is
the 128x64 f-chunk. This computes P = f^T L (shape [64, 2048]) in PSUM.
The epilogue needs f^T [64, 2048] to form de[d] = sum_i P[d,i] * f^T[d,i];
f^T is built once per side with 16 PE transposes (vs streaming 16 MiB of L
twice, transposing f is free).

Sharding: graph b -> core b. Each core streams its two 16 MiB laplacians
through the PE while accumulating P in PSUM; a multiply-reduce produces a
[64, 2] per-core partial. Host finishes the (tiny) masked mean.
"""

import numpy as np

import concourse.bacc as bacc
import concourse.bass as bass
import concourse.mybir as mybir
import concourse.tile as tile
from concourse.bass_utils import run_bass_kernel_spmd

B = 8
N = 2048
D = 64
P = 128
NCHUNK = N // P  # 16 contraction chunks
MM_N = 512       # moving free dim per matmul (PSUM bank limit for f32 out)
NI = N // MM_N   # 4 output column blocks
F32 = mybir.dt.float32
F32R = mybir.dt.float32r

# --- tunables -------------------------------------------------------------
SLAB_ROWS = 512          # rows of L per DMA (must be multiple of 128)
SLAB_BUFS = 3            # slab pool double/triple buffering
# --------------------------------------------------------------------------
NSLAB = N // SLAB_ROWS
N_BLK = SLAB_ROWS // P


def _emit_body(nc, tc, pools, aps):
    (constpool, fpool, fmmpool, ftpool, slabpool, psumpool, smallpool,
     outpool) = pools
    Ls, hs, Lt, ht, ident, out = aps

    ident_sb = constpool.tile([P, P], F32, tag="ident_sb")
    nc.sync.dma_start(out=ident_sb[:], in_=ident[:, :])

    out_sb = outpool.tile([D, 2], F32, tag="out_sb")

    # ---- phase A: h -> f (normalized), f_mm (f32r), fT (via PE transpose)
    f_mms = []
    fT_sbs = []
    for side, h_ap in enumerate((hs, ht)):
        # f_raw[p, k*64+d] = h[k*128+p, d]
        f_raw = fpool.tile([P, NCHUNK * D], F32, tag="f_raw")
        nc.sync.dma_start(
            out=f_raw[:], in_=h_ap.rearrange("(k p) d -> p k d", p=P)
        )

        sq = smallpool.tile([P, NCHUNK * D], F32, tag="sq")
        nc.scalar.square(sq[:], f_raw[:])
        ss = smallpool.tile([P, NCHUNK], F32, tag="ss")
        nc.vector.reduce_sum(
            out=ss[:],
            in_=sq[:].rearrange("p (k d) -> p k d", d=D),
            axis=mybir.AxisListType.X,
        )
        nrm = smallpool.tile([P, NCHUNK], F32, tag="nrm")
        nc.scalar.sqrt(nrm[:], ss[:])
        nc.vector.tensor_scalar_max(nrm[:], nrm[:], 1e-12)
        inv = smallpool.tile([P, NCHUNK], F32, tag="inv")
        nc.vector.reciprocal(inv[:], nrm[:])

        # f = h / max(||h||, eps); f32r copy for the matmul stationary
        f_all = fpool.tile([P, NCHUNK * D], F32, tag="f_all")
        for k in range(NCHUNK):
            nc.scalar.mul(
                f_all[:, k * D : (k + 1) * D],
                f_raw[:, k * D : (k + 1) * D],
                inv[:, k : k + 1],
            )
        f_mm = fmmpool.tile([P, NCHUNK * D], F32R, tag="f_mm")
        nc.vector.tensor_copy(f_mm[:], f_all[:])

        # fT[d, i] = f[i, d] via 16 PE transposes ([128, 64] -> [64, 128]).
        # 4 transposes land in each 2 KiB PSUM bank: start only on the
        # first write to a bank, stop on the last.
        fT_ps = psumpool.tile([D, N], F32, tag="ps", name="fT_ps")
        for k in range(NCHUNK):
            nc.tensor.matmul(
                fT_ps[:, k * P : (k + 1) * P],
                f_all[:, k * D : (k + 1) * D],
                ident_sb[:],
                is_transpose=True,
                start=(k % 4 == 0),
                stop=(k % 4 == 3),
            )
        fT_sb = ftpool.tile([D, N], F32, tag="fT_sb")
        nc.scalar.copy(fT_sb[:], fT_ps[:])
        f_mms.append(f_mm)
        fT_sbs.append(fT_sb)

    # ---- phase B: P = f^T L accumulated over row slabs, then epilogue
    for side, L_ap in enumerate((Ls, Lt)):
        f_mm = f_mms[side]
        fT_sb = fT_sbs[side]
        P_ps = psumpool.tile([D, N], F32, tag="ps", name="P_ps")
        for g in range(NSLAB):
            slab = slabpool.tile([P, N_BLK * N], F32R, tag="slab")
            nc.sync.dma_start(
                out=slab[:],
                in_=L_ap[g * SLAB_ROWS : (g + 1) * SLAB_ROWS, :].rearrange(
                    "(n p) c -> p n c", p=P
                ),
            )
            for n in range(N_BLK):
                j = g * N_BLK + n
                for i in range(NI):
                    nc.tensor.matmul(
                        P_ps[:, i * MM_N : (i + 1) * MM_N],
                        f_mm[:, j * D : (j + 1) * D],
                        slab[:, n * N + i * MM_N : n * N + (i + 1) * MM_N],
                        start=(j == 0),
                        stop=(j == NCHUNK - 1),
                    )

        # de[d] = sum_i fT[d, i] * P[d, i]
        tmp = smallpool.tile([D, N], F32, tag="ttr_tmp")
        nc.vector.tensor_tensor(
            out=tmp[:], in0=P_ps[:], in1=fT_sb[:], op=mybir.AluOpType.mult
        )
        nc.vector.reduce_sum(
            out=out_sb[:, side : side + 1],
            in_=tmp[:],
            axis=mybir.AxisListType.X,
        )

    nc.sync.dma_start(out=out[:], in_=out_sb[:])


def build_program():
    nc = bacc.Bacc(trn_type="TRN2")

    Ls = nc.declare_dram_parameter("Ls", [N, N], F32R, isOutput=False)
    hs = nc.declare_dram_parameter("hs", [N, D], F32, isOutput=False)
    Lt = nc.declare_dram_parameter("Lt", [N, N], F32R, isOutput=False)
    ht = nc.declare_dram_parameter("ht", [N, D], F32, isOutput=False)
    ident = nc.declare_dram_parameter("ident", [P, P], F32, isOutput=False)
    out = nc.declare_dram_parameter("out", [D, 2], F32, isOutput=True)
    aps = (Ls, hs, Lt, ht, ident, out)

    with tile.TileContext(nc) as tc:
        with (
            tc.tile_pool(name="constp", bufs=1) as constpool,
            tc.tile_pool(name="fpool", bufs=2) as fpool,
            tc.tile_pool(name="fmm", bufs=2) as fmmpool,
            tc.tile_pool(name="ftp", bufs=2) as ftpool,
            tc.tile_pool(name="slab", bufs=SLAB_BUFS) as slabpool,
            tc.tile_pool(name="psum", bufs=2, space="PSUM") as psumpool,
            tc.tile_pool(name="small", bufs=2) as smallpool,
            tc.tile_pool(name="outp", bufs=1) as outpool,
        ):
            pools = (constpool, fpool, fmmpool, ftpool, slabpool, psumpool,
                     smallpool, outpool)
            _emit_body(nc, tc, pools, aps)

    nc.compile()
    return nc


_CACHED_NC = None


def _get_nc():
    global _CACHED_NC
    if _CACHED_NC is None:
        _CACHED_NC = build_program()
    return _CACHED_NC


_IDENT = np.eye(P, dtype=np.float32)


def _shard_inputs(inputs):
    lap_s = np.ascontiguousarray(np.asarray(inputs["laplacian_s"], dtype=np.float32))
    lap_t = np.ascontiguousarray(np.asarray(inputs["laplacian_t"], dtype=np.float32))
    h_s = np.ascontiguousarray(np.asarray(inputs["h_s"], dtype=np.float32))
    h_t = np.ascontiguousarray(np.asarray(inputs["h_t"], dtype=np.float32))
    return [
        {
            "Ls": lap_s[b * N : (b + 1) * N],
            "hs": h_s[b * N : (b + 1) * N],
            "Lt": lap_t[b * N : (b + 1) * N],
            "ht": h_t[b * N : (b + 1) * N],
            "ident": _IDENT,
        }
        for b in range(B)
    ]


def _finish(core_outs, inputs):
    has_s = np.asarray(inputs["has_laplacian_s"]).astype(bool)
    has_t = np.asarray(inputs["has_laplacian_t"]).astype(bool)
    d_s = np.empty(B, dtype=np.float64)
    d_t = np.empty(B, dtype=np.float64)
    for b in range(B):
        o = np.asarray(core_outs[b], dtype=np.float64)
        d_s[b] = o[:, 0].sum() / D
        d_t[b] = o[:, 1].sum() / D
    per_graph = 0.5 * (d_s + d_t)
    valid = np.logical_and(has_s, has_t)
    count = valid.sum()
    total = per_graph[valid].sum()
    value = total / max(count, 1.0) if count > 0 else 0.0
    return np.array(value, dtype=np.float32)


def _run(inputs, trace=False, tmpdir=None):
    nc = _get_nc()
    in_maps = _shard_inputs(inputs)
    res = run_bass_kernel_spmd(nc, in_maps, list(range(B)), trace=trace, tmpdir=tmpdir)
    out = _finish([res.results[b]["out"] for b in range(B)], inputs)
    return out, res


def kernel(**inputs):
    out, _ = _run(inputs, trace=False)
    return out


# revision 8
# speedup vs baseline: 2.2770x; 1.1565x over previous
"""DirichletLoss kernel for 8 trn2 NeuronCores.

Math: per graph b, per channel d:
    de[d] = f_d^T L f_d  with f = row-normalized h.

Layout strategy (v2): make L the MOVING matmul operand so each matmul
streams N=512 columns (vs 64 in v1), and use float32r (single-pass fp32,
1 cycle/row at N>=256) instead of 4-pass fp32. The stationary operand is
the 128x64 f-chunk. This computes P = f^T L (shape [64, 2048]) in PSUM.
The epilogue needs f^T [64, 2048] to form de[d] = sum_i P[d,i] * f^T[d,i];
f^T is built once per side with 16 PE transposes (vs streaming 16 MiB of L
twice, transposing f is free).

Sharding: graph b -> core b. Each core streams its two 16 MiB laplacians
through the PE while accumulating P in PSUM; a multiply-reduce produces a
[64, 2] per-core partial. Host finishes the (tiny) masked mean.
"""

import numpy as np

import concourse.bacc as bacc
import concourse.bass as bass
import concourse.mybir as mybir
import concourse.tile as tile
from concourse.bass_utils import run_bass_kernel_spmd

B = 8
N = 2048
D = 64
P = 128
NCHUNK = N // P  # 16 contraction chunks
MM_N = 512       # moving free dim per matmul (PSUM bank limit for f32 out)
NI = N // MM_N   # 4 output column blocks
F32 = mybir.dt.float32
F32R = mybir.dt.float32r

# --- tunables -------------------------------------------------------------
SLAB_BUFS = 3            # slab pool double/triple buffering
# Row counts per DMA slab (each a multiple of 128, summing to N). Side t
# tapers at the end so the last DMA is small: the un-hidden matmul work
# after the final DMA byte is only the last 128-row chunk.
SLABS_S = [512, 512, 512, 512]
SLABS_T = [512, 512, 512, 256, 128, 128]
# --------------------------------------------------------------------------


def _emit_body(nc, tc, pools, aps):
    (constpool, fpool, fmmpool, ftpool, slabpool, psumpool, smallpool,
     outpool) = pools
    Ls, hs, Lt, ht, ident, out = aps

    ident_sb = constpool.tile([P, P], F32, tag="ident_sb")
    nc.sync.dma_start(out=ident_sb[:], in_=ident[:, :])

    out_sb = outpool.tile([D, 2], F32, tag="out_sb")

    # ---- phase A: h -> f (normalized), f_mm (f32r), fT (via PE transpose)
    f_mms = []
    fT_sbs = []
    for side, h_ap in enumerate((hs, ht)):
        # f_raw[p, k*64+d] = h[k*128+p, d]
        f_raw = fpool.tile([P, NCHUNK * D], F32, tag="f_raw")
        nc.sync.dma_start(
            out=f_raw[:], in_=h_ap.rearrange("(k p) d -> p k d", p=P)
        )

        sq = smallpool.tile([P, NCHUNK * D], F32, tag="sq")
        nc.scalar.square(sq[:], f_raw[:])
        ss = smallpool.tile([P, NCHUNK], F32, tag="ss")
        nc.vector.reduce_sum(
            out=ss[:],
            in_=sq[:].rearrange("p (k d) -> p k d", d=D),
            axis=mybir.AxisListType.X,
        )
        nrm = smallpool.tile([P, NCHUNK], F32, tag="nrm")
        nc.scalar.sqrt(nrm[:], ss[:])
        nc.vector.tensor_scalar_max(nrm[:], nrm[:], 1e-12)
        inv = smallpool.tile([P, NCHUNK], F32, tag="inv")
        nc.vector.reciprocal(inv[:], nrm[:])

        # f = h / max(||h||, eps); f32r copy for the matmul stationary
        f_all = fpool.tile([P, NCHUNK * D], F32, tag="f_all")
        for k in range(NCHUNK):
            nc.scalar.mul(
                f_all[:, k * D : (k + 1) * D],
                f_raw[:, k * D : (k + 1) * D],
                inv[:, k : k + 1],
            )
        f_mm = fmmpool.tile([P, NCHUNK * D], F32R, tag="f_mm")
        nc.vector.tensor_copy(f_mm[:], f_all[:])

        # fT[d, i] = f[i, d] via 16 PE transposes ([128, 64] -> [64, 128]).
        # 4 transposes land in each 2 KiB PSUM bank: start only on the
        # first write to a bank, stop on the last.
        fT_ps = psumpool.tile([D, N], F32, tag="ps", name="fT_ps")
        for k in range(NCHUNK):
            nc.tensor.matmul(
                fT_ps[:, k * P : (k + 1) * P],
                f_all[:, k * D : (k + 1) * D],
                ident_sb[:],
                is_transpose=True,
                start=(k % 4 == 0),
                stop=(k % 4 == 3),
            )
        fT_sb = ftpool.tile([D, N], F32, tag="fT_sb")
        nc.scalar.copy(fT_sb[:], fT_ps[:])
        f_mms.append(f_mm)
        fT_sbs.append(fT_sb)

    # ---- phase B: P = f^T L accumulated over row slabs, then epilogue
    for side, (L_ap, slabs) in enumerate(((Ls, SLABS_S), (Lt, SLABS_T))):
        f_mm = f_mms[side]
        fT_sb = fT_sbs[side]
        P_ps = psumpool.tile([D, N], F32, tag="ps", name="P_ps")
        row0 = 0
        for rows in slabs:
            n_blk = rows // P
            slab = slabpool.tile([P, n_blk * N], F32R, tag="slab")
            nc.sync.dma_start(
                out=slab[:],
                in_=L_ap[row0 : row0 + rows, :].rearrange(
                    "(n p) c -> p n c", p=P
                ),
            )
            for n in range(n_blk):
                j = row0 // P + n
                for i in range(NI):
                    nc.tensor.matmul(
                        P_ps[:, i * MM_N : (i + 1) * MM_N],
                        f_mm[:, j * D : (j + 1) * D],
                        slab[:, n * N + i * MM_N : n * N + (i + 1) * MM_N],
                        start=(j == 0),
                        stop=(j == NCHUNK - 1),
                    )
            row0 += rows

        # de[d] = sum_i fT[d, i] * P[d, i], pipelined per PSUM bank so the
        # multiply of bank i starts as soon as its accumulation stops
        # (only DVE can read PSUM for tensor_tensor).
        red4 = smallpool.tile([D, NI], F32, tag="red4")
        for i in range(NI):
            tmp = smallpool.tile([D, MM_N], F32, tag="ttr_tmp")
            nc.vector.tensor_tensor(
                out=tmp[:],
                in0=P_ps[:, i * MM_N : (i + 1) * MM_N],
                in1=fT_sb[:, i * MM_N : (i + 1) * MM_N],
                op=mybir.AluOpType.mult,
            )
            nc.vector.reduce_sum(
                out=red4[:, i : i + 1], in_=tmp[:], axis=mybir.AxisListType.X
            )
        nc.vector.reduce_sum(
            out=out_sb[:, side : side + 1],
            in_=red4[:],
            axis=mybir.AxisListType.X,
        )

    nc.sync.dma_start(out=out[:], in_=out_sb[:])


def build_program():
    nc = bacc.Bacc(trn_type="TRN2")

    Ls = nc.declare_dram_parameter("Ls", [N, N], F32R, isOutput=False)
    hs = nc.declare_dram_parameter("hs", [N, D], F32, isOutput=False)
    Lt = nc.declare_dram_parameter("Lt", [N, N], F32R, isOutput=False)
    ht = nc.declare_dram_parameter("ht", [N, D], F32, isOutput=False)
    ident = nc.declare_dram_parameter("ident", [P, P], F32, isOutput=False)
    out = nc.declare_dram_parameter("out", [D, 2], F32, isOutput=True)
    aps = (Ls, hs, Lt, ht, ident, out)

    with tile.TileContext(nc) as tc:
        with (
            tc.tile_pool(name="constp", bufs=1) as constpool,
            tc.tile_pool(name="fpool", bufs=2) as fpool,
            tc.tile_pool(name="fmm", bufs=2) as fmmpool,
            tc.tile_pool(name="ftp", bufs=2) as ftpool,
            tc.tile_pool(name="slab", bufs=SLAB_BUFS) as slabpool,
            tc.tile_pool(name="psum", bufs=2, space="PSUM") as psumpool,
            tc.tile_pool(name="small", bufs=2) as smallpool,
            tc.tile_pool(name="outp", bufs=1) as outpool,
        ):
            pools = (constpool, fpool, fmmpool, ftpool, slabpool, psumpool,
                     smallpool, outpool)
            _emit_body(nc, tc, pools, aps)

    nc.compile()
    return nc


_CACHED_NC = None


def _get_nc():
    global _CACHED_NC
    if _CACHED_NC is None:
        _CACHED_NC = build_program()
    return _CACHED_NC


_IDENT = np.eye(P, dtype=np.float32)


def _shard_inputs(inputs):
    lap_s = np.ascontiguousarray(np.asarray(inputs["laplacian_s"], dtype=np.float32))
    lap_t = np.ascontiguousarray(np.asarray(inputs["laplacian_t"], dtype=np.float32))
    h_s = np.ascontiguousarray(np.asarray(inputs["h_s"], dtype=np.float32))
    h_t = np.ascontiguousarray(np.asarray(inputs["h_t"], dtype=np.float32))
    return [
        {
            "Ls": lap_s[b * N : (b + 1) * N],
            "hs": h_s[b * N : (b + 1) * N],
            "Lt": lap_t[b * N : (b + 1) * N],
            "ht": h_t[b * N : (b + 1) * N],
            "ident": _IDENT,
        }
        for b in range(B)
    ]


def _finish(core_outs, inputs):
    has_s = np.asarray(inputs["has_laplacian_s"]).astype(bool)
    has_t = np.asarray(inputs["has_laplacian_t"]).astype(bool)
    d_s = np.empty(B, dtype=np.float64)
    d_t = np.empty(B, dtype=np.float64)
    for b in range(B):
        o = np.asarray(core_outs[b], dtype=np.float64)
        d_s[b] = o[:, 0].sum() / D
        d_t[b] = o[:, 1].sum() / D
    per_graph = 0.5 * (d_s + d_t)
    valid = np.logical_and(has_s, has_t)
    count = valid.sum()
    total = per_graph[valid].sum()
    value = total / max(count, 1.0) if count > 0 else 0.0
    return np.array(value, dtype=np.float32)


def _run(inputs, trace=False, tmpdir=None):
    nc = _get_nc()
    in_maps = _shard_inputs(inputs)
    res = run_bass_kernel_spmd(nc, in_maps, list(range(B)), trace=trace, tmpdir=tmpdir)
    out = _finish([res.results[b]["out"] for b in range(B)], inputs)
    return out, res


def kernel(**inputs):
    out, _ = _run(inputs, trace=False)
    return out
